# revision 1
# baseline (speedup 1.0000x reference)
"""ARNet forward (teacher forcing) as a Trainium2 Bass kernel.

out[b, i] = sum_j w[j] * seq[b, i+j],  seq = concat(x, true_output[:, :63], axis=1)
          = (seq @ T)[b, i]            with T[k, i] = w[k-i] (Toeplitz, [127, 64])

Sharding: pure data parallel over the batch dim across 8 NeuronCores.

The rel-err budget (2e-2) leaves ample room for bf16 (measured ~3e-3), which
halves HBM traffic vs fp32 AND runs the PE at full rate (fp32 matmul streams
at quarter rate and was the baseline bottleneck at 86% PE busy).

Device-side work per core (125000 rows):
  - Host builds seqT blocks [30, 128, 4096] bf16 (+ a [128, 3072] tail): row
    k (<127) = seq position k, column r = batch row (blk*4096 + r); row 127
    pad (cols >= 125000 pad). Each block is a fully contiguous 1MB DRAM
    region: the HWDGE only fans a DMA across all 16 SDMA engines for
    128-partition tiles with contiguous sources (a strided [127, N] source
    ran on ONE engine at 27GB/s).
  - Per block: one 1MB input DMA, 8 matmuls [127,64]x[127,512] -> PSUM
    [128,1024] tiles holding 4 chunks each (even chunks at partitions 0-63,
    odd at 64-127 via matmul tile_position col 64 - the Toeplitz loads into
    PE columns 64-127), 2 full-width PSUM->SBUF copies with fp32->bf16 cast
    (alternating DVE/ACT), one 512KB output DMA using all DMA ports.
    Small grains + deep pools keep both DMA rings saturated at the ~435GB/s
    SBUF-fabric ceiling with no compute-gated stalls in ramp or drain.
  - Toeplitz lhsT is the only stationary; weights reload is a 64-col
    background load hidden by the 512-col matmuls.
"""

import sys

if "/opt/trn_rl_repo" not in sys.path:
    sys.path.insert(0, "/opt/trn_rl_repo")

import ml_dtypes
import numpy as np

import concourse.bacc as bacc
import concourse.mybir as mybir
import concourse.tile as tile
from concourse.bass_utils import run_bass_kernel_spmd

B = 1_000_000
N_LAGS = 64
NF = 64
SEQ = N_LAGS + NF - 1  # 127
N_CORES = 8
RPC = B // N_CORES  # 125000 rows per core

CHUNK = 512  # rows per matmul (= PSUM bank in fp32)
NCHUNKS = 246  # ceil(125000/512) rounded up to even (computed chunks)
CPB = 8  # chunks per full block
NBLK = 30  # full blocks; tail block has 6 chunks
TAILC = NCHUNKS - NBLK * CPB  # 6
BLKCOLS = CPB * CHUNK  # 4096
TAILCOLS = TAILC * CHUNK  # 3072
OUT_COLS = (NCHUNKS // 2) * CHUNK  # 62976... see below

F32 = mybir.dt.float32
BF16 = mybir.dt.bfloat16
NP_BF16 = ml_dtypes.bfloat16

# column layout of the packed output: global pair J = 2*blk + t (tail J=60+t),
# out[h*64 + i, J*1024 + e*512 + s] = y[(4J + 2h + e)*512 + s, i]
NPAIRJ = NBLK * CPB // 4 + TAILC // 4 + (1 if TAILC % 4 else 0)  # 62
OUT_COLS = NPAIRJ * 2 * CHUNK  # 63488

_cache = {}


def _build_nc():
    nc = bacc.Bacc("TRN2", target_bir_lowering=False, debug=False, num_devices=N_CORES)
    sqt = nc.dram_tensor("sqt", [NBLK, 128, BLKCOLS], BF16, kind="ExternalInput")
    sqt_t = nc.dram_tensor("sqt_t", [128, TAILCOLS], BF16, kind="ExternalInput")
    tpl = nc.dram_tensor("tpl", [128, NF], BF16, kind="ExternalInput")
    out = nc.dram_tensor("out", [128, OUT_COLS], BF16, kind="ExternalOutput")

    with tile.TileContext(nc) as tc:
        with (
            tc.tile_pool(name="consts", bufs=1) as consts,
            tc.tile_pool(name="sqin", bufs=18) as spool,
            tc.tile_pool(name="oout", bufs=12) as opool,
            tc.tile_pool(name="psO", bufs=4, space="PSUM") as psO,
        ):
            tpl_sb = consts.tile([128, NF], BF16)
            nc.sync.dma_start(tpl_sb[:], tpl.ap())
            # tail block's input, prefetched at start so the final compute
            # chain never waits on the last-arriving DMA
            s_tail = consts.tile([128, TAILCOLS], BF16)
            nc.sync.dma_start(s_tail[:], sqt_t.ap())

            for b in range(NBLK + 1):
                is_tail = b == NBLK
                nch = TAILC if is_tail else CPB
                if is_tail:
                    s_t = s_tail
                else:
                    s_t = spool.tile([128, BLKCOLS], BF16, tag="sqin")
                    nc.sync.dma_start(s_t[:], sqt.ap()[b])
                o_t = opool.tile([128, (CPB // 2) * CHUNK], BF16, tag="oout")
                for t in range((nch + 3) // 4):
                    ps = psO.tile([128, 2 * CHUNK], F32, tag="psO")
                    # e-first order so the e=0 half-copy fires while e=1 runs
                    for e in range(2):
                        for h in range(2):
                            c = 4 * t + 2 * h + e
                            if c < nch:
                                nc.tensor.matmul(
                                    ps[h * 64 : h * 64 + 64, e * CHUNK : (e + 1) * CHUNK],
                                    tpl_sb[0:SEQ, :],
                                    s_t[0:SEQ, c * CHUNK : (c + 1) * CHUNK],
                                    start=True,
                                    stop=True,
                                )
                        psrc = ps[:, e * CHUNK : (e + 1) * CHUNK]
                        dst = o_t[:, t * 1024 + e * CHUNK : t * 1024 + (e + 1) * CHUNK]
                        if e == 0:
                            nc.vector.tensor_copy(dst, psrc)
                        else:
                            nc.scalar.copy(dst, psrc)
                ocols = ((nch + 3) // 4) * 1024  # cols actually written
                nc.scalar.dma_start(
                    out.ap()[:, b * 2048 : b * 2048 + ocols], o_t[:, 0:ocols]
                )
    nc.compile()
    return nc


def _get_nc():
    if "nc" not in _cache:
        _cache["nc"] = _build_nc()
    return _cache["nc"]


def _prepare_in_maps(x, true_output, w):
    xb = np.asarray(x, dtype=np.float32).astype(NP_BF16)
    tob = np.asarray(true_output, dtype=np.float32).astype(NP_BF16)
    w = np.asarray(w, dtype=np.float32).reshape(N_LAGS)

    tpl = np.zeros((128, NF), np.float32)
    for i in range(NF):
        tpl[i : i + N_LAGS, i] = w
    tpl = tpl.astype(NP_BF16)

    cols = NBLK * BLKCOLS + TAILCOLS  # 125952
    flat = np.zeros((N_CORES, SEQ, cols), NP_BF16)
    for c in range(N_CORES):
        rows = slice(c * RPC, (c + 1) * RPC)
        flat[c, :N_LAGS, :RPC] = xb[rows].T
        flat[c, N_LAGS:, :RPC] = tob[rows, : NF - 1].T
    sqt = np.zeros((N_CORES, NBLK, 128, BLKCOLS), NP_BF16)
    sqt[:, :, :SEQ, :] = (
        flat[:, :, : NBLK * BLKCOLS]
        .reshape(N_CORES, SEQ, NBLK, BLKCOLS)
        .swapaxes(1, 2)
    )
    sqt_t = np.zeros((N_CORES, 128, TAILCOLS), NP_BF16)
    sqt_t[:, :SEQ, :] = flat[:, :, NBLK * BLKCOLS :]

    return [
        {"sqt": sqt[c], "sqt_t": sqt_t[c], "tpl": tpl} for c in range(N_CORES)
    ]


def _decode_out(results):
    outs = []
    for r in results:
        oh = np.asarray(r["out"]).reshape(2, 64, NPAIRJ, 2, CHUNK)  # h,i,J,e,s
        full = oh.transpose(2, 0, 3, 4, 1).reshape(NPAIRJ * 4 * CHUNK, NF)
        outs.append(full[:RPC].astype(np.float32))
    return np.concatenate(outs, axis=0)


def kernel(x, true_output, w):
    nc = _get_nc()
    in_maps = _prepare_in_maps(x, true_output, w)
    res = run_bass_kernel_spmd(nc, in_maps, core_ids=list(range(N_CORES)))
    return _decode_out(res.results)


def run_traced(x, true_output, w, tmpdir=None):
    """Like kernel() but captures an NTFF profile; returns (out, BassKernelResults)."""
    import types

    import antenv
    import concourse.bass_utils as bass_utils

    if "antenv.axon_hooks" not in sys.modules:
        hooks_mod = types.ModuleType("antenv.axon_hooks")
        _hook = [None]
        hooks_mod.set_axon_ntff_profile_hook = lambda h: _hook.__setitem__(0, h)
        hooks_mod.get_axon_ntff_profile_hook = lambda: _hook[0]
        sys.modules["antenv.axon_hooks"] = hooks_mod
        antenv.axon_hooks = hooks_mod
        from trn_agent_boot.trn_boot import _ntff_profile_via_ctypes

        hooks_mod.set_axon_ntff_profile_hook(
            _ntff_profile_via_ctypes("/opt/axon/libaxon_pjrt.so")
        )
    bass_utils.upload_artifacts = lambda d: d  # no S3 in this container

    if tmpdir is not None:
        import shutil

        shutil.rmtree(tmpdir, ignore_errors=True)

    nc = _get_nc()
    in_maps = _prepare_in_maps(x, true_output, w)
    res = run_bass_kernel_spmd(
        nc, in_maps, core_ids=list(range(N_CORES)), trace=True, tmpdir=tmpdir
    )
    return _decode_out(res.results), res

